# revision 4
# baseline (speedup 1.0000x reference)
"""Trainium2 Bass kernel for nn_MoELayer (top-1 MoE, dense-masked reference).

Strategy
--------
The reference runs every expert's MLP over every token and then keeps only
the output of each token's argmax-gated expert.  Mathematically the output
for token t is exactly `mlp_{top1(t)}(x_t)`, so we:

  1. compute the (tiny) gate + argmax on the host in float64,
  2. group tokens by chosen expert (expert-parallel sharding: core e gets
     expert e's weights and the tokens routed to it, padded to a fixed
     capacity C),
  3. run one dense MLP per core on its token batch:
        yT = W2^T @ relu(W1^T @ xT + b1) + b2      (both matmuls produce
     transposed outputs so no on-device transposes are needed; host
     supplies x and the weights pre-tiled into the SBUF layout),
  4. scatter the per-expert outputs back into the full [B,T,D] tensor.

This does 1/E of the reference FLOPs.  Matmuls run in bfloat16: the PE
streams bf16 at the same 1 column/cycle as fp32r, but bf16 weights use the
separate LDWEIGHTS path with fast-weight-load, which pipelines under the
previous matmul's streaming instead of serializing a 4-byte weight load
into every matmul the way fp32/fp32r does.  The full hidden activation
(H x C, bf16) stays resident in SBUF, so the H-reduction of the second
matmul is a single PSUM accumulation group per output tile -- no vector
engine adds, and one activation-engine eviction per output tile.
bf16 end-to-end relative error is ~4e-4 rms, well inside tolerance.
"""

import os
import sys
import time

import numpy as np

for _p in ("/opt/trn_rl_repo", "/root/.axon_site/_ro/trn_rl_repo"):
    if os.path.isdir(_p) and _p not in sys.path:
        sys.path.insert(0, _p)

import concourse.bass as bass
import concourse.bacc as bacc
import concourse.mybir as mybir
from concourse.bass_utils import run_bass_kernel_spmd
from concourse.tile import TileContext

# run_bass_kernel_spmd's trace path (BASS_TRACE=1) imports antenv.axon_hooks,
# which not every container ships; force tracing off when it's absent so a
# stray env var can't crash the run.
try:
    from antenv.axon_hooks import get_axon_ntff_profile_hook  # noqa: F401
except Exception:
    os.environ["BASS_NEVER_TRACE"] = "1"

B, T, D, H, E = 4, 2048, 1024, 4096, 8
BT = B * T
N_CORES = 8
F32 = mybir.dt.float32
BF16 = mybir.dt.bfloat16
BF16NP = mybir.dt.np(mybir.dt.bfloat16)
AF = mybir.ActivationFunctionType

ND = D // 128   # 8 tiles of the d axis
NHT = H // 128  # 32 tiles of the h axis

_PROGRAM_CACHE: dict[int, bass.Bass] = {}
LAST_RESULT = None  # BassKernelResults of the most recent device run (for test.py)


def _token_tiles(C):
    """Split C tokens into matmul moving-dim tiles, each in [256, 512]."""
    assert C >= 512 and C % 8 == 0
    tiles = []
    t0 = 0
    rem = C
    while rem >= 768:
        tiles.append((t0, 512))
        t0 += 512
        rem -= 512
    if rem > 512:
        tiles.append((t0, rem - 256))
        t0 += rem - 256
        rem = 256
    tiles.append((t0, rem))
    return tiles


def _build_program(C: int, repeats: int = 1) -> bass.Bass:
    """One expert MLP over C tokens: yT[D,C] = W2^T @ relu(W1^T @ xT + b1) + b2.

    All 8 cores run this same program on different data (SPMD).

    Host-side input layouts (partition dim first, bf16 unless noted):
      xT  [128, ND*C]    xT[p, dc*C + t]          = x[t, dc*128 + p]
      w1  [128, NHT*1024] w1[p, hs*1024+dc*128+c] = W1[dc*128 + p, hs*128 + c]
      w2  [128, NHT*1024] w2[p, hs*1024 + d]      = W2[hs*128 + p, d]
      b1c [128, NHT] f32  b1c[p, g]               = b1[g*128 + p]
      b2c [128, ND] f32   b2c[p, g]               = b2[g*128 + p]

    `repeats` re-runs the whole (idempotent) compute body that many times
    inside one NEFF — used only by test.py to amplify kernel time above the
    axon per-execution launch overhead when measuring.
    """
    nc = bacc.Bacc("TRN2", target_bir_lowering=False, debug=False)

    xT = nc.dram_tensor("xT", [128, ND * C], BF16, kind="ExternalInput").ap()
    w1 = nc.dram_tensor("w1", [128, NHT * 1024], BF16, kind="ExternalInput").ap()
    b1c = nc.dram_tensor("b1c", [128, NHT], F32, kind="ExternalInput").ap()
    w2 = nc.dram_tensor("w2", [128, NHT * 1024], BF16, kind="ExternalInput").ap()
    b2c = nc.dram_tensor("b2c", [128, ND], F32, kind="ExternalInput").ap()
    yT = nc.dram_tensor("yT", [D, C], F32, kind="ExternalOutput").ap()

    ttiles = _token_tiles(C)

    with TileContext(nc) as tc:
        with (
            tc.tile_pool(name="const", bufs=1) as constp,
            tc.tile_pool(name="resid", bufs=1) as resid,
            tc.tile_pool(name="w1p", bufs=4) as w1p,
            tc.tile_pool(name="outp", bufs=4) as outp,
            tc.tile_pool(name="psA", bufs=4, space="PSUM") as psA,
            tc.tile_pool(name="psB", bufs=4, space="PSUM") as psB,
        ):
            # Warmup ACT with no cross-engine waits: walrus's lower_act
            # attaches the one-time activation-table load to the first ACT
            # instruction, consuming one of its two sync-wait slots. Give it
            # a dependency-free instruction so real ACTs keep both slots.
            warm = constp.tile([128, 1], F32, tag="warm")
            nc.scalar.memzero(warm[:, :])
            nc.scalar.activation(warm[:, :], warm[:, :], AF.Relu)
            nc.scalar.activation(warm[:, :], warm[:, :], AF.Identity)

            # Biases first (tiny), then x (needed by the very first matmul
            # group), then W2 (only needed once phase B starts, but large —
            # give it the head start on its own queues).
            b1t = constp.tile([128, NHT], F32, tag="b1t")
            nc.sync.dma_start(b1t[:, :], b1c)
            b2t = constp.tile([128, ND], F32, tag="b2t")
            nc.sync.dma_start(b2t[:, :], b2c)

            xt = resid.tile([128, ND * C], BF16, tag="xt")
            for dc in range(ND):
                nc.sync.dma_start(
                    xt[:, dc * C : (dc + 1) * C], xT[:, dc * C : (dc + 1) * C]
                )
            w2t = resid.tile([128, NHT * 1024], BF16, tag="w2t")
            for q in range(4):
                nc.sync.dma_start(
                    w2t[:, q * 8192 : (q + 1) * 8192],
                    w2[:, q * 8192 : (q + 1) * 8192],
                )

            # Full hidden activation resident: col block hs holds
            # relu(W1^T x + b1)[hs*128:(hs+1)*128, :] in bf16.
            ht = resid.tile([128, NHT * C], BF16, tag="ht")

            for _ in range(repeats):
                # Phase A: ht[hs] = relu(W1(:,hs)^T @ x + b1[hs])
                for hs in range(NHT):
                    w1t = w1p.tile([128, 1024], BF16, tag="w1c")
                    nc.sync.dma_start(w1t[:, :], w1[:, hs * 1024 : (hs + 1) * 1024])
                    for t0, tn in ttiles:
                        ps = psA.tile([128, 512], F32, tag="psA")
                        for dc in range(ND):
                            nc.tensor.matmul(
                                ps[:, :tn],
                                w1t[:, dc * 128 : (dc + 1) * 128],
                                xt[:, dc * C + t0 : dc * C + t0 + tn],
                                start=(dc == 0),
                                stop=(dc == ND - 1),
                            )
                        nc.scalar.activation(
                            ht[:, hs * C + t0 : hs * C + t0 + tn],
                            ps[:, :tn],
                            AF.Relu,
                            bias=b1t[:, hs : hs + 1],
                        )

                # Phase B: yT[dt] = W2(:,dt)^T @ ht + b2[dt], one PSUM
                # accumulation group over all 32 h tiles per output tile.
                for dt in range(ND):
                    for t0, tn in ttiles:
                        ps = psB.tile([128, 512], F32, tag="psB")
                        for hs in range(NHT):
                            nc.tensor.matmul(
                                ps[:, :tn],
                                w2t[:, hs * 1024 + dt * 128 : hs * 1024 + dt * 128 + 128],
                                ht[:, hs * C + t0 : hs * C + t0 + tn],
                                start=(hs == 0),
                                stop=(hs == NHT - 1),
                            )
                        ot = outp.tile([128, 512], F32, tag="ot")
                        nc.scalar.activation(
                            ot[:, :tn], ps[:, :tn], AF.Identity, bias=b2t[:, dt : dt + 1]
                        )
                        nc.sync.dma_start(
                            yT[dt * 128 : (dt + 1) * 128, t0 : t0 + tn], ot[:, :tn]
                        )

    nc.compile()
    return nc


def _get_program(C: int) -> bass.Bass:
    if C not in _PROGRAM_CACHE:
        _PROGRAM_CACHE[C] = _build_program(C)
    return _PROGRAM_CACHE[C]


def _pack_x(xe, C):
    """[C, D] f32 -> [128, ND*C] bf16 with xT[p, dc*C + t] = xe[t, dc*128+p]."""
    xTt = xe.T.astype(BF16NP).reshape(ND, 128, C)
    return np.ascontiguousarray(xTt.transpose(1, 0, 2).reshape(128, ND * C))


def _prepare(x, Wg, bg, W1, b1, W2, b2):
    """Host routing: fp64 gate + argmax, group tokens by expert, build the
    per-core (per-expert) input maps padded to capacity C."""
    xf = np.ascontiguousarray(np.asarray(x, dtype=np.float32).reshape(BT, D))

    # Host gate in float64: scores are tiny (BT x E) and fp64 argmax is
    # robust to any fp32 accumulation-order noise in the reference.
    scores = xf.astype(np.float64) @ np.asarray(Wg, dtype=np.float64)
    scores += np.asarray(bg, dtype=np.float64)
    top1 = np.argmax(scores, axis=-1)

    counts = np.bincount(top1, minlength=E)
    # 8-aligned capacity (32B DMA lines). Above ~1600 the resident x/h
    # tiles exceed the SBUF budget, so larger routing skews fall back to
    # multiple passes (never hit for the ~1k-per-expert counts this gate
    # produces).
    C = max(512, int(np.ceil(counts.max() / 8)) * 8)
    C = min(C, 1344)

    W1f = np.asarray(W1, dtype=np.float32)
    b1f = np.asarray(b1, dtype=np.float32)
    W2f = np.asarray(W2, dtype=np.float32)
    b2f = np.asarray(b2, dtype=np.float32)

    in_maps = []
    idxs = []
    for e in range(E):
        idx = np.nonzero(top1 == e)[0]
        idxs.append(idx)
        xe = np.zeros((C, D), dtype=np.float32)
        xe[: min(len(idx), C)] = xf[idx[:C]]
        w1p = (
            W1f[e]
            .astype(BF16NP)
            .reshape(ND, 128, NHT, 128)
            .transpose(1, 2, 0, 3)
            .reshape(128, NHT * 1024)
        )
        w2p = (
            W2f[e]
            .astype(BF16NP)
            .reshape(NHT, 128, D)
            .transpose(1, 0, 2)
            .reshape(128, NHT * 1024)
        )
        in_maps.append(
            {
                "xT": _pack_x(xe, C),
                "w1": np.ascontiguousarray(w1p),
                "b1c": np.ascontiguousarray(b1f[e].reshape(NHT, 128).T),
                "w2": np.ascontiguousarray(w2p),
                "b2c": np.ascontiguousarray(b2f[e].reshape(ND, 128).T),
            }
        )
    return C, in_maps, idxs


_FASTPATH_CACHE: dict[int, object] = {}


def _make_fastpath(nc):
    """Memoized version of run_bass_kernel_spmd's axon execution path: the
    same sharded custom-call jit, kept alive so repeat kernel() calls skip
    jax retracing and NEFF reload. Numerically identical machinery."""
    import jax
    from jax.sharding import Mesh, PartitionSpec
    from jax.experimental.shard_map import shard_map
    from concourse.bass2jax import (
        _bass_exec_p,
        install_neuronx_cc_hook,
        partition_id_tensor,
    )

    install_neuronx_cc_hook()
    partition_name = nc.partition_id_tensor.name if nc.partition_id_tensor else None
    in_names, out_names, out_avals = [], [], []
    for alloc in nc.m.functions[0].allocations:
        if not isinstance(alloc, mybir.MemoryLocationSet):
            continue
        name = alloc.memorylocations[0].name
        if alloc.kind == "ExternalInput":
            if name != partition_name:
                in_names.append(name)
        elif alloc.kind == "ExternalOutput":
            out_names.append(name)
            out_avals.append(
                jax.core.ShapedArray(tuple(alloc.tensor_shape), mybir.dt.np(alloc.dtype))
            )
    all_names = in_names + out_names + ([partition_name] if partition_name else [])

    def _body(*args):
        operands = list(args)
        if partition_name is not None:
            operands.append(partition_id_tensor())
        return tuple(
            _bass_exec_p.bind(
                *operands,
                out_avals=tuple(out_avals),
                in_names=tuple(all_names),
                out_names=tuple(out_names),
                lowering_input_output_aliases=(),
                sim_require_finite=True,
                sim_require_nnan=True,
                nc=nc,
            )
        )

    mesh = Mesh(np.asarray(jax.devices()[:N_CORES]), ("core",))
    nin, nout = len(in_names), len(out_names)
    fn = jax.jit(
        shard_map(
            _body,
            mesh=mesh,
            in_specs=(PartitionSpec("core"),) * (nin + nout),
            out_specs=(PartitionSpec("core"),) * nout,
            check_rep=False,
        )
    )

    def run(in_maps):
        args = [
            np.concatenate([np.asarray(m[nm]) for m in in_maps], axis=0)
            for nm in in_names
        ]
        for aval in out_avals:
            args.append(np.zeros((N_CORES * aval.shape[0], *aval.shape[1:]), aval.dtype))
        outs = fn(*args)
        return [
            {
                nm: np.asarray(outs[i]).reshape(N_CORES, *out_avals[i].shape)[c]
                for i, nm in enumerate(out_names)
            }
            for c in range(N_CORES)
        ]

    return run


def _run_spmd(C, nc, in_maps):
    global LAST_RESULT
    # Transient tunnel/device errors (e.g. NRT_EXEC_UNIT_UNRECOVERABLE on a
    # shared terminal) are retried with a fresh executable.
    for attempt in range(3):
        try:
            if C in _FASTPATH_CACHE:
                return _FASTPATH_CACHE[C](in_maps)
            # First call per capacity: the prescribed run_bass_kernel_spmd
            # path (compiles the NEFF); then build the memoized executable
            # for repeat calls.
            res = run_bass_kernel_spmd(nc, in_maps, list(range(N_CORES)))
            LAST_RESULT = res
            try:
                _FASTPATH_CACHE[C] = _make_fastpath(nc)
            except Exception:
                pass
            return res.results
        except Exception:
            if attempt == 2:
                raise
            _FASTPATH_CACHE.pop(C, None)
            time.sleep(2.0)


def kernel(x, Wg, bg, W1, b1, W2, b2):
    C, in_maps, idxs = _prepare(x, Wg, bg, W1, b1, W2, b2)
    nc = _get_program(C)
    results = _run_spmd(C, nc, in_maps)

    out = np.empty((BT, D), dtype=np.float32)
    for e in range(E):
        n_e = min(len(idxs[e]), C)
        if n_e:
            out[idxs[e][:n_e]] = results[e]["yT"][:, :n_e].T

    # Overflow passes: only if some expert drew more than C (=1344) tokens,
    # which this gate's near-uniform routing never does for the given data.
    max_count = max(len(i) for i in idxs)
    done = C
    while done < max_count:
        xf = np.ascontiguousarray(np.asarray(x, dtype=np.float32).reshape(BT, D))
        for e in range(E):
            idx = idxs[e][done : done + C]
            xe = np.zeros((C, D), dtype=np.float32)
            xe[: len(idx)] = xf[idx]
            in_maps[e]["xT"] = _pack_x(xe, C)
        results = _run_spmd(C, nc, in_maps)
        for e in range(E):
            idx = idxs[e][done : done + C]
            if len(idx):
                out[idx] = results[e]["yT"][:, : len(idx)].T
        done += C
    return out.reshape(B, T, D)


# revision 5
# speedup vs baseline: 1.0204x; 1.0204x over previous
"""Trainium2 Bass kernel for nn_MoELayer (top-1 MoE, dense-masked reference).

Strategy
--------
The reference runs every expert's MLP over every token and then keeps only
the output of each token's argmax-gated expert.  Mathematically the output
for token t is exactly `mlp_{top1(t)}(x_t)`, so we:

  1. compute the (tiny) gate + argmax on the host in float64,
  2. group tokens by chosen expert (expert-parallel sharding: core e gets
     expert e's weights and the tokens routed to it, padded to a fixed
     capacity C),
  3. run one dense MLP per core on its token batch:
        yT = W2^T @ relu(W1^T @ xT + b1) + b2      (both matmuls produce
     transposed outputs so no on-device transposes are needed; host
     supplies x and the weights pre-tiled into the SBUF layout),
  4. scatter the per-expert outputs back into the full [B,T,D] tensor.

This does 1/E of the reference FLOPs.  Matmuls run in bfloat16: the PE
streams bf16 at the same 1 column/cycle as fp32r, but bf16 weights use the
separate LDWEIGHTS path with fast-weight-load, which pipelines under the
previous matmul's streaming instead of serializing a 4-byte weight load
into every matmul the way fp32/fp32r does.  The full hidden activation
(H x C, bf16) stays resident in SBUF, so the H-reduction of the second
matmul is a single PSUM accumulation group per output tile -- no vector
engine adds, and one activation-engine eviction per output tile.
bf16 end-to-end relative error is ~4e-4 rms, well inside tolerance.
"""

import os
import sys
import time

import numpy as np

for _p in ("/opt/trn_rl_repo", "/root/.axon_site/_ro/trn_rl_repo"):
    if os.path.isdir(_p) and _p not in sys.path:
        sys.path.insert(0, _p)

import concourse.bass as bass
import concourse.bacc as bacc
import concourse.mybir as mybir
from concourse.bass_utils import run_bass_kernel_spmd
from concourse.tile import TileContext

# run_bass_kernel_spmd's trace path (BASS_TRACE=1) imports antenv.axon_hooks,
# which not every container ships; force tracing off when it's absent so a
# stray env var can't crash the run.
try:
    from antenv.axon_hooks import get_axon_ntff_profile_hook  # noqa: F401
except Exception:
    os.environ["BASS_NEVER_TRACE"] = "1"

B, T, D, H, E = 4, 2048, 1024, 4096, 8
BT = B * T
N_CORES = 8
F32 = mybir.dt.float32
BF16 = mybir.dt.bfloat16
BF16NP = mybir.dt.np(mybir.dt.bfloat16)
AF = mybir.ActivationFunctionType

ND = D // 128   # 8 tiles of the d axis
NHT = H // 128  # 32 tiles of the h axis

_PROGRAM_CACHE: dict[int, bass.Bass] = {}
LAST_RESULT = None  # BassKernelResults of the most recent device run (for test.py)


def _token_tiles(C):
    """Split C tokens into matmul moving-dim tiles, each in [256, 512]."""
    assert C >= 512 and C % 8 == 0
    tiles = []
    t0 = 0
    rem = C
    while rem >= 768:
        tiles.append((t0, 512))
        t0 += 512
        rem -= 512
    if rem > 512:
        tiles.append((t0, rem - 256))
        t0 += rem - 256
        rem = 256
    tiles.append((t0, rem))
    return tiles


def _build_program(C: int, repeats: int = 1) -> bass.Bass:
    """One expert MLP over C tokens: yT[D,C] = W2^T @ relu(W1^T @ xT + b1) + b2.

    All 8 cores run this same program on different data (SPMD).

    Host-side input layouts (partition dim first, bf16 unless noted):
      xT  [128, ND*C]    xT[p, dc*C + t]          = x[t, dc*128 + p]
      w1  [128, NHT*1024] w1[p, hs*1024+dc*128+c] = W1[dc*128 + p, hs*128 + c]
      w2  [128, NHT*1024] w2[p, hs*1024 + d]      = W2[hs*128 + p, d]
      b1c [128, NHT] f32  b1c[p, g]               = b1[g*128 + p]
      b2c [128, ND] f32   b2c[p, g]               = b2[g*128 + p]

    `repeats` re-runs the whole (idempotent) compute body that many times
    inside one NEFF — used only by test.py to amplify kernel time above the
    axon per-execution launch overhead when measuring.
    """
    nc = bacc.Bacc("TRN2", target_bir_lowering=False, debug=False)

    xT = nc.dram_tensor("xT", [128, ND * C], BF16, kind="ExternalInput").ap()
    w1 = nc.dram_tensor("w1", [128, NHT * 1024], BF16, kind="ExternalInput").ap()
    b1c = nc.dram_tensor("b1c", [128, NHT], F32, kind="ExternalInput").ap()
    w2 = nc.dram_tensor("w2", [128, NHT * 1024], BF16, kind="ExternalInput").ap()
    b2c = nc.dram_tensor("b2c", [128, ND], F32, kind="ExternalInput").ap()
    yT = nc.dram_tensor("yT", [D, C], F32, kind="ExternalOutput").ap()

    ttiles = _token_tiles(C)

    with TileContext(nc) as tc:
        with (
            tc.tile_pool(name="const", bufs=1) as constp,
            tc.tile_pool(name="resid", bufs=1) as resid,
            tc.tile_pool(name="w1p", bufs=4) as w1p,
            tc.tile_pool(name="outp", bufs=4) as outp,
            # 6/2 split: phase A groups are 8 MMs (~1.7us) so deeper
            # pipelining helps there; phase B groups are 32 MMs (~7us), the
            # ACT eviction (~0.7us) keeps up easily with 2 banks.
            tc.tile_pool(name="psA", bufs=6, space="PSUM") as psA,
            tc.tile_pool(name="psB", bufs=2, space="PSUM") as psB,
        ):
            # Warmup ACT with no cross-engine waits: walrus's lower_act
            # attaches the one-time activation-table load to the first ACT
            # instruction, consuming one of its two sync-wait slots. Give it
            # a dependency-free instruction so real ACTs keep both slots.
            warm = constp.tile([128, 1], F32, tag="warm")
            nc.scalar.memzero(warm[:, :])
            nc.scalar.activation(warm[:, :], warm[:, :], AF.Relu)
            nc.scalar.activation(warm[:, :], warm[:, :], AF.Identity)

            # Biases first (tiny), then x (needed by the very first matmul
            # group), then W2 (only needed once phase B starts, but large —
            # give it the head start on its own queues).
            b1t = constp.tile([128, NHT], F32, tag="b1t")
            nc.sync.dma_start(b1t[:, :], b1c)
            b2t = constp.tile([128, ND], F32, tag="b2t")
            nc.sync.dma_start(b2t[:, :], b2c)

            xt = resid.tile([128, ND * C], BF16, tag="xt")
            for dc in range(ND):
                nc.sync.dma_start(
                    xt[:, dc * C : (dc + 1) * C], xT[:, dc * C : (dc + 1) * C]
                )
            w2t = resid.tile([128, NHT * 1024], BF16, tag="w2t")
            for q in range(4):
                nc.sync.dma_start(
                    w2t[:, q * 8192 : (q + 1) * 8192],
                    w2[:, q * 8192 : (q + 1) * 8192],
                )

            # Full hidden activation resident: col block hs holds
            # relu(W1^T x + b1)[hs*128:(hs+1)*128, :] in bf16.
            ht = resid.tile([128, NHT * C], BF16, tag="ht")

            for _ in range(repeats):
                # Phase A: ht[hs] = relu(W1(:,hs)^T @ x + b1[hs])
                for hs in range(NHT):
                    w1t = w1p.tile([128, 1024], BF16, tag="w1c")
                    nc.sync.dma_start(w1t[:, :], w1[:, hs * 1024 : (hs + 1) * 1024])
                    for t0, tn in ttiles:
                        ps = psA.tile([128, 512], F32, tag="psA")
                        for dc in range(ND):
                            nc.tensor.matmul(
                                ps[:, :tn],
                                w1t[:, dc * 128 : (dc + 1) * 128],
                                xt[:, dc * C + t0 : dc * C + t0 + tn],
                                start=(dc == 0),
                                stop=(dc == ND - 1),
                            )
                        nc.scalar.activation(
                            ht[:, hs * C + t0 : hs * C + t0 + tn],
                            ps[:, :tn],
                            AF.Relu,
                            bias=b1t[:, hs : hs + 1],
                        )

                # Phase B: yT[dt] = W2(:,dt)^T @ ht + b2[dt], one PSUM
                # accumulation group over all 32 h tiles per output tile.
                for dt in range(ND):
                    for t0, tn in ttiles:
                        ps = psB.tile([128, 512], F32, tag="psB")
                        for hs in range(NHT):
                            nc.tensor.matmul(
                                ps[:, :tn],
                                w2t[:, hs * 1024 + dt * 128 : hs * 1024 + dt * 128 + 128],
                                ht[:, hs * C + t0 : hs * C + t0 + tn],
                                start=(hs == 0),
                                stop=(hs == NHT - 1),
                            )
                        ot = outp.tile([128, 512], F32, tag="ot")
                        nc.scalar.activation(
                            ot[:, :tn], ps[:, :tn], AF.Identity, bias=b2t[:, dt : dt + 1]
                        )
                        nc.sync.dma_start(
                            yT[dt * 128 : (dt + 1) * 128, t0 : t0 + tn], ot[:, :tn]
                        )

    nc.compile()
    return nc


def _get_program(C: int) -> bass.Bass:
    if C not in _PROGRAM_CACHE:
        _PROGRAM_CACHE[C] = _build_program(C)
    return _PROGRAM_CACHE[C]


def _pack_x(xe, C):
    """[C, D] f32 -> [128, ND*C] bf16 with xT[p, dc*C + t] = xe[t, dc*128+p]."""
    xTt = xe.T.astype(BF16NP).reshape(ND, 128, C)
    return np.ascontiguousarray(xTt.transpose(1, 0, 2).reshape(128, ND * C))


def _prepare(x, Wg, bg, W1, b1, W2, b2):
    """Host routing: fp64 gate + argmax, group tokens by expert, build the
    per-core (per-expert) input maps padded to capacity C."""
    xf = np.ascontiguousarray(np.asarray(x, dtype=np.float32).reshape(BT, D))

    # Host gate in float64: scores are tiny (BT x E) and fp64 argmax is
    # robust to any fp32 accumulation-order noise in the reference.
    scores = xf.astype(np.float64) @ np.asarray(Wg, dtype=np.float64)
    scores += np.asarray(bg, dtype=np.float64)
    top1 = np.argmax(scores, axis=-1)

    counts = np.bincount(top1, minlength=E)
    # 8-aligned capacity (32B DMA lines). Above ~1600 the resident x/h
    # tiles exceed the SBUF budget, so larger routing skews fall back to
    # multiple passes (never hit for the ~1k-per-expert counts this gate
    # produces).
    C = max(512, int(np.ceil(counts.max() / 8)) * 8)
    C = min(C, 1344)

    W1f = np.asarray(W1, dtype=np.float32)
    b1f = np.asarray(b1, dtype=np.float32)
    W2f = np.asarray(W2, dtype=np.float32)
    b2f = np.asarray(b2, dtype=np.float32)

    in_maps = []
    idxs = []
    for e in range(E):
        idx = np.nonzero(top1 == e)[0]
        idxs.append(idx)
        xe = np.zeros((C, D), dtype=np.float32)
        xe[: min(len(idx), C)] = xf[idx[:C]]
        w1p = (
            W1f[e]
            .astype(BF16NP)
            .reshape(ND, 128, NHT, 128)
            .transpose(1, 2, 0, 3)
            .reshape(128, NHT * 1024)
        )
        w2p = (
            W2f[e]
            .astype(BF16NP)
            .reshape(NHT, 128, D)
            .transpose(1, 0, 2)
            .reshape(128, NHT * 1024)
        )
        in_maps.append(
            {
                "xT": _pack_x(xe, C),
                "w1": np.ascontiguousarray(w1p),
                "b1c": np.ascontiguousarray(b1f[e].reshape(NHT, 128).T),
                "w2": np.ascontiguousarray(w2p),
                "b2c": np.ascontiguousarray(b2f[e].reshape(ND, 128).T),
            }
        )
    return C, in_maps, idxs


_FASTPATH_CACHE: dict[int, object] = {}


def _make_fastpath(nc):
    """Memoized version of run_bass_kernel_spmd's axon execution path: the
    same sharded custom-call jit, kept alive so repeat kernel() calls skip
    jax retracing and NEFF reload. Numerically identical machinery."""
    import jax
    from jax.sharding import Mesh, PartitionSpec
    from jax.experimental.shard_map import shard_map
    from concourse.bass2jax import (
        _bass_exec_p,
        install_neuronx_cc_hook,
        partition_id_tensor,
    )

    install_neuronx_cc_hook()
    partition_name = nc.partition_id_tensor.name if nc.partition_id_tensor else None
    in_names, out_names, out_avals = [], [], []
    for alloc in nc.m.functions[0].allocations:
        if not isinstance(alloc, mybir.MemoryLocationSet):
            continue
        name = alloc.memorylocations[0].name
        if alloc.kind == "ExternalInput":
            if name != partition_name:
                in_names.append(name)
        elif alloc.kind == "ExternalOutput":
            out_names.append(name)
            out_avals.append(
                jax.core.ShapedArray(tuple(alloc.tensor_shape), mybir.dt.np(alloc.dtype))
            )
    all_names = in_names + out_names + ([partition_name] if partition_name else [])

    def _body(*args):
        operands = list(args)
        if partition_name is not None:
            operands.append(partition_id_tensor())
        return tuple(
            _bass_exec_p.bind(
                *operands,
                out_avals=tuple(out_avals),
                in_names=tuple(all_names),
                out_names=tuple(out_names),
                lowering_input_output_aliases=(),
                sim_require_finite=True,
                sim_require_nnan=True,
                nc=nc,
            )
        )

    mesh = Mesh(np.asarray(jax.devices()[:N_CORES]), ("core",))
    nin, nout = len(in_names), len(out_names)
    fn = jax.jit(
        shard_map(
            _body,
            mesh=mesh,
            in_specs=(PartitionSpec("core"),) * (nin + nout),
            out_specs=(PartitionSpec("core"),) * nout,
            check_rep=False,
        )
    )

    def run(in_maps):
        args = [
            np.concatenate([np.asarray(m[nm]) for m in in_maps], axis=0)
            for nm in in_names
        ]
        for aval in out_avals:
            args.append(np.zeros((N_CORES * aval.shape[0], *aval.shape[1:]), aval.dtype))
        outs = fn(*args)
        return [
            {
                nm: np.asarray(outs[i]).reshape(N_CORES, *out_avals[i].shape)[c]
                for i, nm in enumerate(out_names)
            }
            for c in range(N_CORES)
        ]

    return run


def _run_spmd(C, nc, in_maps):
    global LAST_RESULT
    # Transient tunnel/device errors (e.g. NRT_EXEC_UNIT_UNRECOVERABLE on a
    # shared terminal) are retried with a fresh executable.
    for attempt in range(3):
        try:
            if C in _FASTPATH_CACHE:
                return _FASTPATH_CACHE[C](in_maps)
            # First call per capacity: the prescribed run_bass_kernel_spmd
            # path (compiles the NEFF); then build the memoized executable
            # for repeat calls.
            res = run_bass_kernel_spmd(nc, in_maps, list(range(N_CORES)))
            LAST_RESULT = res
            try:
                _FASTPATH_CACHE[C] = _make_fastpath(nc)
            except Exception:
                pass
            return res.results
        except Exception:
            if attempt == 2:
                raise
            _FASTPATH_CACHE.pop(C, None)
            time.sleep(2.0)


def kernel(x, Wg, bg, W1, b1, W2, b2):
    C, in_maps, idxs = _prepare(x, Wg, bg, W1, b1, W2, b2)
    nc = _get_program(C)
    results = _run_spmd(C, nc, in_maps)

    out = np.empty((BT, D), dtype=np.float32)
    for e in range(E):
        n_e = min(len(idxs[e]), C)
        if n_e:
            out[idxs[e][:n_e]] = results[e]["yT"][:, :n_e].T

    # Overflow passes: only if some expert drew more than C (=1344) tokens,
    # which this gate's near-uniform routing never does for the given data.
    max_count = max(len(i) for i in idxs)
    done = C
    while done < max_count:
        xf = np.ascontiguousarray(np.asarray(x, dtype=np.float32).reshape(BT, D))
        for e in range(E):
            idx = idxs[e][done : done + C]
            xe = np.zeros((C, D), dtype=np.float32)
            xe[: len(idx)] = xf[idx]
            in_maps[e]["xT"] = _pack_x(xe, C)
        results = _run_spmd(C, nc, in_maps)
        for e in range(E):
            idx = idxs[e][done : done + C]
            if len(idx):
                out[idx] = results[e]["yT"][:, : len(idx)].T
        done += C
    return out.reshape(B, T, D)
